# revision 16
# baseline (speedup 1.0000x reference)
"""MANN (phase-blended mixture-of-experts) forward pass on 8 Trainium2 cores.

Strategy (data-parallel, per sharding hint):
  - Shard batch B=512 across 8 cores (64 samples each); replicate all weights.
  - Host-side prep: transpose expert weights to [K, IN, OUT] (so the device
    streams them in natural layout with the contraction dim on partitions),
    pad layer-1 input dim 480 -> 512, pre-gather the gating columns.
  - Device: activations kept transposed [feat, B].  Key algebraic trick:
        y = sum_k g[:,k] * (x @ Wk[k].T)  ==  sum_k ((g[:,k]*x) @ Wk[k].T)
    so scaling the stationary activations by g[:,k] lets all 8 experts x 4
    K-subtiles accumulate into a single PSUM tile per layer.  The blended
    bias g @ bk is one extra small matmul into the same PSUM group.
  - ELU built from primitives: elu(x) = max(x, exp(min(x,0)) - 1).

Modes (MANN_MM_MODE env var, default bf16):
  fp32  - exact, but fp32 matmul is 4 cycles/row on PE
  fp32r - fp32 bits, float32r matmul dtype (1 cycle/row for N>=256)
  bf16  - weights cast to bf16 on host (halves DMA traffic; ~4e-3 rel err)
"""

import json
import os

import numpy as np
import ml_dtypes

import concourse.bass as bass
import concourse.bass2jax as bass2jax
import concourse.mybir as mybir
import concourse.tile as tile
from concourse import bass_utils as _bass_utils
from concourse.bass_utils import run_bass_kernel_spmd
from concourse.masks import make_identity


def _legalize_bir(bir_bytes):
    """This container's walrus build rejects instructions carrying more than
    one semaphore wait (setupSyncWait: "Too many sync wait commands" -- hit by
    the Tile kernel-tail Drain).  Equivalent legal form: hoist all but one
    wait onto single-wait NoOps immediately preceding the instruction on the
    same engine (sequencers process waits in program order)."""
    data = json.loads(bir_bytes)
    n = 0
    for fn in data.get("functions", []):
        for bb in fn.get("blocks", []):
            out = []
            for inst in bb.get("instructions", []):
                si = inst.get("sync_info")
                waits = si.get("on_wait", []) if si else []
                if len(waits) > 1:
                    for w in waits[:-1]:
                        n += 1
                        out.append({
                            "debug": inst.get("debug", 0),
                            "engine": inst["engine"],
                            "ins": [], "outs": [],
                            "name": f"I-mwfix-{n}",
                            "opcode": "NoOp",
                            "sync_info": {"on_update": [], "on_wait": [w]},
                        })
                    si["on_wait"] = [waits[-1]]
                out.append(inst)
            bb["instructions"] = out
    return json.dumps(data).encode()


_orig_compile_bir_kernel = _bass_utils.compile_bir_kernel


def _patched_compile_bir_kernel(bir_json, tmpdir, neff_name="file.neff"):
    return _orig_compile_bir_kernel(_legalize_bir(bir_json), tmpdir,
                                    neff_name=neff_name)


bass2jax.compile_bir_kernel = _patched_compile_bir_kernel
_bass_utils.compile_bir_kernel = _patched_compile_bir_kernel

B, IN_DIM, OUT_DIM, HID, K, GH, NG = 512, 480, 400, 512, 8, 128, 32
N_CORES = 8
BS = B // N_CORES  # 64 samples per core
IN_PAD = 512       # layer-1 contraction dim padded to 4x128
KSUB = 4           # 512 / 128 contraction subtiles (all layers, post-pad)
OUTS = (HID, HID, OUT_DIM)
P = 128

MM_MODE = os.environ.get("MANN_MM_MODE", "bf16")

# Set to the BassKernelResults of the last run (for test harnesses).
LAST_RESULTS = None

_NC_CACHE = {}


def _elu_from(nc, pool, src_ap, out_shape, tag):
    """elu(src) = max(src, min(exp(src), 1) - 1); src may be PSUM or SBUF.
    3 ops, exp directly from src (activations here are small enough that
    exp cannot overflow fp32).  Returns a new SBUF fp32 tile."""
    f32 = mybir.dt.float32
    texp = pool.tile(out_shape, f32, tag=f"{tag}_exp")
    nc.scalar.activation(texp, src_ap, mybir.ActivationFunctionType.Exp)
    nc.vector.tensor_scalar(texp, texp, 1.0, -1.0, mybir.AluOpType.min,
                            mybir.AluOpType.add)
    y = pool.tile(out_shape, f32, tag=f"{tag}_y")
    nc.vector.tensor_tensor(y, src_ap, texp, mybir.AluOpType.max)
    return y


def _build(mode, repeat=1):
    f32 = mybir.dt.float32
    if mode == "bf16":
        wdt = mybir.dt.bfloat16
        mmdt = mybir.dt.bfloat16
    elif mode == "fp32r":
        wdt = f32
        mmdt = mybir.dt.float32r
    else:
        wdt = f32
        mmdt = f32

    def mm_ap(ap):
        return ap.bitcast(mmdt) if mode == "fp32r" else ap

    nc = bass.Bass()

    xT_d = nc.dram_tensor("xT", [IN_PAD, BS], f32, kind="ExternalInput")
    ginT_d = nc.dram_tensor("ginT", [NG, BS], f32, kind="ExternalInput")
    w_d = [
        nc.dram_tensor(f"w{l}", [K, IN_PAD if l == 0 else HID, OUTS[l]], wdt,
                       kind="ExternalInput")
        for l in range(3)
    ]
    b_d = [
        nc.dram_tensor(f"b{l}", [K, OUTS[l]], wdt, kind="ExternalInput")
        for l in range(3)
    ]
    gw1_d = nc.dram_tensor("gw1", [NG, GH], f32, kind="ExternalInput")
    gw2_d = nc.dram_tensor("gw2", [GH, GH], f32, kind="ExternalInput")
    gw3_d = nc.dram_tensor("gw3", [GH, K], f32, kind="ExternalInput")
    gb1_d = nc.dram_tensor("gb1", [GH, 1], f32, kind="ExternalInput")
    gb2_d = nc.dram_tensor("gb2", [GH, 1], f32, kind="ExternalInput")
    gb3_d = nc.dram_tensor("gb3", [K, 1], f32, kind="ExternalInput")
    # E[j, e*128 + p] = (j == e): replicates g row e across 128 partitions
    # via matmul E_slice.T @ gT.
    emat_d = nc.dram_tensor("emat", [K, K * P], f32, kind="ExternalInput")
    out_d = nc.dram_tensor("out", [BS, OUT_DIM], f32, kind="ExternalOutput")

    w_bufs = int(os.environ.get("MANN_W_BUFS", "24" if mode == "bf16" else "12"))
    with tile.TileContext(nc) as tc:
        with (
            tc.tile_pool(name="consts", bufs=1) as cpool,
            tc.tile_pool(name="w", bufs=w_bufs) as wpool,
            tc.tile_pool(name="stat", bufs=2) as spool,
            tc.tile_pool(name="xt", bufs=2) as xpool,
            tc.tile_pool(name="y", bufs=2) as ypool,
            tc.tile_pool(name="psy", bufs=2, space="PSUM") as pspool,
            tc.tile_pool(name="pstr", bufs=2, space="PSUM") as ptpool,
            tc.tile_pool(name="psg", bufs=1, space="PSUM") as pgpool,
        ):
            pools = (cpool, wpool, spool, xpool, ypool, pspool, ptpool, pgpool)

            # ---- constants ----
            xt0 = cpool.tile([P, KSUB, BS], f32)
            nc.sync.dma_start(xt0, xT_d.rearrange("(ko p) b -> p ko b", p=P))
            gin = cpool.tile([NG, BS], f32)
            nc.sync.dma_start(gin, ginT_d[:])
            gw1 = cpool.tile([NG, GH], f32)
            nc.sync.dma_start(gw1, gw1_d[:])
            gw2 = cpool.tile([GH, GH], f32)
            nc.sync.dma_start(gw2, gw2_d[:])
            gw3 = cpool.tile([GH, K], f32)
            nc.sync.dma_start(gw3, gw3_d[:])
            gb1 = cpool.tile([GH, 1], f32)
            nc.sync.dma_start(gb1, gb1_d[:])
            gb2 = cpool.tile([GH, 1], f32)
            nc.sync.dma_start(gb2, gb2_d[:])
            gb3 = cpool.tile([K, 1], f32)
            nc.sync.dma_start(gb3, gb3_d[:])
            emat = cpool.tile([K, K * P], f32)
            nc.sync.dma_start(emat, emat_d[:])
            bts = []
            for l in range(3):
                bt = cpool.tile([K, OUTS[l]], wdt, tag=f"b{l}")
                nc.sync.dma_start(bt, b_d[l][:])
                bts.append(bt)
            ident = cpool.tile([BS, BS], f32)
            make_identity(nc, ident)
            consts = (xt0, gin, gw1, gw2, gw3, gb1, gb2, gb3, emat, bts, ident)

            if repeat == 0:
                # no-op baseline for dispatch-overhead measurement
                yo = ypool.tile([BS, OUT_DIM], f32, tag="yo")
                nc.vector.memset(yo, 0.0)
                nc.sync.dma_start(out_d[:], yo)
            for _rep in range(repeat):
                _emit_body(nc, mode, mmdt, mm_ap, wdt, pools, w_d, b_d, out_d,
                           consts, accum=(_rep > 0))

    return nc


def _emit_body(nc, mode, mmdt, mm_ap, wdt, pools, w_d, b_d, out_d, consts,
               accum=False):
    f32 = mybir.dt.float32
    cpool, wpool, spool, xpool, ypool, pspool, ptpool, pgpool = pools
    xt0, gin, gw1, gw2, gw3, gb1, gb2, gb3, emat, bts, ident = consts

    # ---- weight slab DMAs, issued first (DMA is the bottleneck) ----
    wsl = []
    for l in range(3):
        row = []
        for e in range(K):
            t = wpool.tile([P, KSUB, OUTS[l]], wdt, tag="w")
            nc.sync.dma_start(
                t[:, :, : OUTS[l]],
                w_d[l][e].rearrange("(ko p) n -> p ko n", p=P),
            )
            row.append(t)
        wsl.append(row)

    # ---- gating MLP (fp32, exact) ----
    pg1 = pgpool.tile([GH, BS], f32, tag="psg")
    nc.tensor.matmul(pg1, lhsT=gw1, rhs=gin, start=True, stop=True)
    zg1 = ypool.tile([GH, BS], f32, tag="zg1")
    nc.scalar.activation(zg1, pg1, mybir.ActivationFunctionType.Identity,
                         bias=gb1)
    h1 = _elu_from(nc, ypool, zg1, [GH, BS], "g1")

    pg2 = pgpool.tile([GH, BS], f32, tag="psg")
    nc.tensor.matmul(pg2, lhsT=gw2, rhs=h1, start=True, stop=True)
    zg2 = ypool.tile([GH, BS], f32, tag="zg2")
    nc.scalar.activation(zg2, pg2, mybir.ActivationFunctionType.Identity,
                         bias=gb2)
    h2 = _elu_from(nc, ypool, zg2, [GH, BS], "g2")

    pg3 = pgpool.tile([K, BS], f32, tag="psg")
    nc.tensor.matmul(pg3, lhsT=gw3, rhs=h2, start=True, stop=True)
    gT = ypool.tile([K, BS], f32, tag="gT")
    nc.scalar.activation(gT, pg3, mybir.ActivationFunctionType.Identity,
                         bias=gb3)
    if mode == "bf16":
        gT_mm = ypool.tile([K, BS], mmdt, tag="gTmm")
        nc.vector.tensor_copy(gT_mm, gT)
    else:
        gT_mm = gT

    # replicate g across partitions: gTb[p, e, b] = g[b, e]
    pgt = pgpool.tile([P, K, BS], f32, tag="psgtb")
    for e in range(K):
        nc.tensor.matmul(pgt[:, e, :], lhsT=emat[:, e * P:(e + 1) * P],
                         rhs=gT, start=True, stop=True)
    gTb = ypool.tile([P, K, BS], f32, tag="gTb")
    nc.vector.tensor_copy(gTb, pgt)

    # ---- motion layers ----
    # Each layer's output columns are split into two halves so the DVE/ACT
    # post-processing (ELU) and PE transposes of half 0 overlap the PE
    # matmuls of half 1.
    xt = xt0
    sdt = mmdt if mode == "bf16" else f32
    for l in range(3):
        outl = OUTS[l]
        halves = [(0, 256), (256, outl)]

        # per-expert scaled stationaries: one broadcast mult per (expert,
        # k-half) instead of 32 tiny mults
        xk = spool.tile([P, K, KSUB, BS], sdt, tag="xk")
        for e in range(K):
            gslab = gTb[:, e:e + 1, :].to_broadcast((P, 2, BS))
            nc.vector.tensor_tensor(xk[:, e, 0:2, :], xt[:, 0:2, :], gslab,
                                    mybir.AluOpType.mult)
            nc.vector.tensor_tensor(xk[:, e, 2:4, :], xt[:, 2:4, :], gslab,
                                    mybir.AluOpType.mult)

        use_pair = os.environ.get("MANN_PAIR", "1") == "1"
        pss = []
        for h, (lo, hi) in enumerate(halves):
            if use_pair:
                # Two experts run concurrently in disjoint 64-col groups of
                # the PE array (even experts -> psum rows 0:64, odd ->
                # 64:128 via tile_position=(0,64)); summed on DVE after.
                ps_full = pspool.tile([2 * BS, 256], f32, tag=f"psy{h}",
                                      name=f"psy{l}_{h}")
                psA = ps_full[0:BS, : hi - lo]
                psB = ps_full[BS:2 * BS, : hi - lo]
                nc.tensor.matmul(psA, lhsT=mm_ap(gT_mm),
                                 rhs=mm_ap(bts[l][:, lo:hi]),
                                 start=True, stop=False,
                                 skip_group_check=True)
                for e0 in range(0, K, 2):
                    for ks in range(KSUB):
                        last = (e0 == K - 2 and ks == KSUB - 1)
                        nc.tensor.matmul(
                            psA,
                            lhsT=mm_ap(xk[:, e0, ks, :]),
                            rhs=mm_ap(wsl[l][e0][:, ks, lo:hi]),
                            start=False, stop=last,
                            skip_group_check=True,
                        )
                        nc.tensor.matmul(
                            psB,
                            lhsT=mm_ap(xk[:, e0 + 1, ks, :]),
                            rhs=mm_ap(wsl[l][e0 + 1][:, ks, lo:hi]),
                            start=(e0 == 0 and ks == 0), stop=last,
                            tile_position=(0, BS),
                            skip_group_check=True,
                        )
                pss.append((psA, psB))
            else:
                ps_full = pspool.tile([BS, 256], f32, tag=f"psy{h}",
                                      name=f"psy{l}_{h}")
                ps = ps_full[:, : hi - lo]
                nc.tensor.matmul(ps, lhsT=mm_ap(gT_mm),
                                 rhs=mm_ap(bts[l][:, lo:hi]),
                                 start=True, stop=False)
                for e in range(K):
                    for ks in range(KSUB):
                        nc.tensor.matmul(
                            ps,
                            lhsT=mm_ap(xk[:, e, ks, :]),
                            rhs=mm_ap(wsl[l][e][:, ks, lo:hi]),
                            start=False,
                            stop=(e == K - 1 and ks == KSUB - 1),
                        )
                pss.append((ps, None))

        if l < 2:
            ptr = ptpool.tile([P, KSUB, BS], f32, tag="ptr")
            xt_next = xpool.tile([P, KSUB, BS], f32, tag="xtn")
            for h, (lo, hi) in enumerate(halves):
                psA, psB = pss[h]
                if psB is not None:
                    z = ypool.tile([BS, hi - lo], f32, tag=f"z{h}")
                    nc.vector.tensor_tensor(z, psA, psB,
                                            mybir.AluOpType.add)
                    src = z
                else:
                    src = psA
                y = _elu_from(nc, ypool, src, [BS, hi - lo], f"ml{h}")
                for c in range(2):
                    nc.tensor.transpose(ptr[:, 2 * h + c, :],
                                        y[:, c * P:(c + 1) * P], ident)
                nc.vector.tensor_copy(xt_next[:, 2 * h:2 * h + 2, :],
                                      ptr[:, 2 * h:2 * h + 2, :])
            xt = xt_next
        else:
            yo = ypool.tile([BS, OUT_DIM], f32, tag="yo")
            for h, (lo, hi) in enumerate(halves):
                psA, psB = pss[h]
                if psB is not None:
                    nc.vector.tensor_tensor(yo[:, lo:hi], psA, psB,
                                            mybir.AluOpType.add)
                else:
                    nc.vector.tensor_copy(yo[:, lo:hi], psA)
            if accum:
                # benchmark-repeat builds accumulate so no body is dead code
                nc.gpsimd.dma_start(out_d[:], yo,
                                    accum_op=mybir.AluOpType.add)
            else:
                nc.sync.dma_start(out_d[:], yo)


def _get_nc(mode):
    repeat = int(os.environ.get("MANN_BENCH_REPEAT", "1"))
    key = (mode, repeat)
    if key not in _NC_CACHE:
        _NC_CACHE[key] = _build(mode, repeat)
    return _NC_CACHE[key]


def _make_emat():
    e = np.zeros((K, K * P), np.float32)
    for j in range(K):
        e[j, j * P:(j + 1) * P] = 1.0
    return e


def prepare_inputs(x, gating_idx, GW1, Gb1, GW2, Gb2, GW3, Gb3,
                   Wk1, bk1, Wk2, bk2, Wk3, bk3, mode):
    wnp = ml_dtypes.bfloat16 if mode == "bf16" else np.float32
    f32 = np.float32
    x = np.asarray(x, f32)
    idx = np.asarray(gating_idx).astype(np.int64)

    xT = np.zeros((IN_PAD, B), f32)
    xT[:IN_DIM] = x.T
    ginT = np.ascontiguousarray(x[:, idx].T)

    w1 = np.zeros((K, IN_PAD, HID), f32)
    w1[:, :IN_DIM] = np.asarray(Wk1, f32).transpose(0, 2, 1)
    w2 = np.ascontiguousarray(np.asarray(Wk2, f32).transpose(0, 2, 1))
    w3 = np.ascontiguousarray(np.asarray(Wk3, f32).transpose(0, 2, 1))

    shared = {
        "w0": w1.astype(wnp), "w1": w2.astype(wnp), "w2": w3.astype(wnp),
        "b0": np.asarray(bk1, f32).astype(wnp),
        "b1": np.asarray(bk2, f32).astype(wnp),
        "b2": np.asarray(bk3, f32).astype(wnp),
        "gw1": np.asarray(GW1, f32), "gw2": np.asarray(GW2, f32),
        "gw3": np.asarray(GW3, f32),
        "gb1": np.asarray(Gb1, f32).reshape(GH, 1),
        "gb2": np.asarray(Gb2, f32).reshape(GH, 1),
        "gb3": np.asarray(Gb3, f32).reshape(K, 1),
        "emat": _make_emat(),
    }
    in_maps = []
    for c in range(N_CORES):
        m = dict(shared)
        m["xT"] = np.ascontiguousarray(xT[:, c * BS:(c + 1) * BS])
        m["ginT"] = np.ascontiguousarray(ginT[:, c * BS:(c + 1) * BS])
        in_maps.append(m)
    return in_maps


def kernel(**inputs):
    global LAST_RESULTS
    mode = MM_MODE
    nc = _get_nc(mode)
    in_maps = prepare_inputs(mode=mode, **inputs)
    trace = os.environ.get("MANN_TRACE", "0") == "1"
    kwargs = {}
    if trace:
        kwargs["trace"] = True
    res = run_bass_kernel_spmd(nc, in_maps, core_ids=list(range(N_CORES)),
                               **kwargs)
    LAST_RESULTS = res
    out = np.concatenate([r["out"] for r in res.results], axis=0)
    return out.astype(np.float32)


# revision 18
# speedup vs baseline: 1.5387x; 1.5387x over previous
"""MANN (phase-blended mixture-of-experts) forward pass on 8 Trainium2 cores.

Strategy (data-parallel, per sharding hint):
  - Shard batch B=512 across 8 cores (64 samples each); replicate all weights.
  - Host-side prep: transpose expert weights to [K, IN, OUT] (so the device
    streams them in natural layout with the contraction dim on partitions),
    pad layer-1 input dim 480 -> 512, pre-gather the gating columns.
  - Device: activations kept transposed [feat, B].  Key algebraic trick:
        y = sum_k g[:,k] * (x @ Wk[k].T)  ==  sum_k ((g[:,k]*x) @ Wk[k].T)
    so scaling the stationary activations by g[:,k] lets all 8 experts x 4
    K-subtiles accumulate into a single PSUM tile per layer.  The blended
    bias g @ bk is one extra small matmul into the same PSUM group.
  - ELU built from primitives: elu(x) = max(x, exp(min(x,0)) - 1).

Modes (MANN_MM_MODE env var, default fp32):
  fp32  - exact (rel err ~6e-7); ~30 us/core measured; DMA/PE balanced
  bf16  - weights cast to bf16 on host: halves DMA traffic and runs matmuls
          at 1 cycle/row (~15 us/core) at ~3.5e-3 max rel err.  Flip the env
          var if the accuracy budget allows.
(float32r was investigated and rejected: walrus requires an fp32->fp32r
data conversion that the host API does not expose.)
"""

import json
import os

import numpy as np
import ml_dtypes

import concourse.bass as bass
import concourse.bass2jax as bass2jax
import concourse.mybir as mybir
import concourse.tile as tile
from concourse import bass_utils as _bass_utils
from concourse.bass_utils import run_bass_kernel_spmd
from concourse.masks import make_identity


def _legalize_bir(bir_bytes):
    """This container's walrus build rejects instructions carrying more than
    one semaphore wait (setupSyncWait: "Too many sync wait commands" -- hit by
    the Tile kernel-tail Drain).  Equivalent legal form: hoist all but one
    wait onto single-wait NoOps immediately preceding the instruction on the
    same engine (sequencers process waits in program order)."""
    data = json.loads(bir_bytes)
    n = 0
    for fn in data.get("functions", []):
        for bb in fn.get("blocks", []):
            out = []
            for inst in bb.get("instructions", []):
                si = inst.get("sync_info")
                waits = si.get("on_wait", []) if si else []
                if len(waits) > 1:
                    for w in waits[:-1]:
                        n += 1
                        out.append({
                            "debug": inst.get("debug", 0),
                            "engine": inst["engine"],
                            "ins": [], "outs": [],
                            "name": f"I-mwfix-{n}",
                            "opcode": "NoOp",
                            "sync_info": {"on_update": [], "on_wait": [w]},
                        })
                    si["on_wait"] = [waits[-1]]
                out.append(inst)
            bb["instructions"] = out
    return json.dumps(data).encode()


_orig_compile_bir_kernel = _bass_utils.compile_bir_kernel


def _patched_compile_bir_kernel(bir_json, tmpdir, neff_name="file.neff"):
    return _orig_compile_bir_kernel(_legalize_bir(bir_json), tmpdir,
                                    neff_name=neff_name)


bass2jax.compile_bir_kernel = _patched_compile_bir_kernel
_bass_utils.compile_bir_kernel = _patched_compile_bir_kernel

B, IN_DIM, OUT_DIM, HID, K, GH, NG = 512, 480, 400, 512, 8, 128, 32
N_CORES = 8
BS = B // N_CORES  # 64 samples per core
IN_PAD = 512       # layer-1 contraction dim padded to 4x128
KSUB = 4           # 512 / 128 contraction subtiles (all layers, post-pad)
OUTS = (HID, HID, OUT_DIM)
P = 128

MM_MODE = os.environ.get("MANN_MM_MODE", "fp32")

# Set to the BassKernelResults of the last run (for test harnesses).
LAST_RESULTS = None

_NC_CACHE = {}


def _elu_from(nc, pool, src_ap, out_shape, tag):
    """elu(src) = max(src, min(exp(src), 1) - 1); src may be PSUM or SBUF.
    3 ops, exp directly from src (activations here are small enough that
    exp cannot overflow fp32).  Returns a new SBUF fp32 tile."""
    f32 = mybir.dt.float32
    texp = pool.tile(out_shape, f32, tag=f"{tag}_exp")
    nc.scalar.activation(texp, src_ap, mybir.ActivationFunctionType.Exp)
    nc.vector.tensor_scalar(texp, texp, 1.0, -1.0, mybir.AluOpType.min,
                            mybir.AluOpType.add)
    y = pool.tile(out_shape, f32, tag=f"{tag}_y")
    nc.vector.tensor_tensor(y, src_ap, texp, mybir.AluOpType.max)
    return y


def _build(mode, repeat=1):
    f32 = mybir.dt.float32
    if mode == "bf16":
        wdt = mybir.dt.bfloat16
        mmdt = mybir.dt.bfloat16
    else:
        wdt = f32
        mmdt = f32

    def mm_ap(ap):
        return ap

    nc = bass.Bass()

    xT_d = nc.dram_tensor("xT", [IN_PAD, BS], f32, kind="ExternalInput")
    ginT_d = nc.dram_tensor("ginT", [NG, BS], f32, kind="ExternalInput")
    w_d = [
        nc.dram_tensor(f"w{l}", [K, IN_PAD if l == 0 else HID, OUTS[l]], wdt,
                       kind="ExternalInput")
        for l in range(3)
    ]
    b_d = [
        nc.dram_tensor(f"b{l}", [K, OUTS[l]], wdt, kind="ExternalInput")
        for l in range(3)
    ]
    gw1_d = nc.dram_tensor("gw1", [NG, GH], f32, kind="ExternalInput")
    gw2_d = nc.dram_tensor("gw2", [GH, GH], f32, kind="ExternalInput")
    gw3_d = nc.dram_tensor("gw3", [GH, K], f32, kind="ExternalInput")
    gb1_d = nc.dram_tensor("gb1", [GH, 1], f32, kind="ExternalInput")
    gb2_d = nc.dram_tensor("gb2", [GH, 1], f32, kind="ExternalInput")
    gb3_d = nc.dram_tensor("gb3", [K, 1], f32, kind="ExternalInput")
    # E[j, e*128 + p] = (j == e): replicates g row e across 128 partitions
    # via matmul E_slice.T @ gT.
    emat_d = nc.dram_tensor("emat", [K, K * P], f32, kind="ExternalInput")
    out_d = nc.dram_tensor("out", [BS, OUT_DIM], f32, kind="ExternalOutput")

    w_bufs = int(os.environ.get("MANN_W_BUFS", "24" if mode == "bf16" else "12"))
    with tile.TileContext(nc) as tc:
        with (
            tc.tile_pool(name="consts", bufs=1) as cpool,
            tc.tile_pool(name="w", bufs=w_bufs) as wpool,
            tc.tile_pool(name="stat", bufs=2) as spool,
            tc.tile_pool(name="xt", bufs=2) as xpool,
            tc.tile_pool(name="y", bufs=2) as ypool,
            tc.tile_pool(name="psy", bufs=2, space="PSUM") as pspool,
            tc.tile_pool(name="pstr", bufs=2, space="PSUM") as ptpool,
            tc.tile_pool(name="psg", bufs=1, space="PSUM") as pgpool,
        ):
            pools = (cpool, wpool, spool, xpool, ypool, pspool, ptpool, pgpool)

            # ---- constants ----
            xt0 = cpool.tile([P, KSUB, BS], f32)
            nc.sync.dma_start(xt0, xT_d.rearrange("(ko p) b -> p ko b", p=P))
            gin = cpool.tile([NG, BS], f32)
            nc.sync.dma_start(gin, ginT_d[:])
            gw1 = cpool.tile([NG, GH], f32)
            nc.sync.dma_start(gw1, gw1_d[:])
            gw2 = cpool.tile([GH, GH], f32)
            nc.sync.dma_start(gw2, gw2_d[:])
            gw3 = cpool.tile([GH, K], f32)
            nc.sync.dma_start(gw3, gw3_d[:])
            gb1 = cpool.tile([GH, 1], f32)
            nc.sync.dma_start(gb1, gb1_d[:])
            gb2 = cpool.tile([GH, 1], f32)
            nc.sync.dma_start(gb2, gb2_d[:])
            gb3 = cpool.tile([K, 1], f32)
            nc.sync.dma_start(gb3, gb3_d[:])
            emat = cpool.tile([K, K * P], f32)
            nc.sync.dma_start(emat, emat_d[:])
            bts = []
            for l in range(3):
                bt = cpool.tile([K, OUTS[l]], wdt, tag=f"b{l}")
                nc.sync.dma_start(bt, b_d[l][:])
                bts.append(bt)
            ident = cpool.tile([BS, BS], f32)
            make_identity(nc, ident)
            consts = (xt0, gin, gw1, gw2, gw3, gb1, gb2, gb3, emat, bts, ident)

            if repeat == 0:
                # no-op baseline for dispatch-overhead measurement
                yo = ypool.tile([BS, OUT_DIM], f32, tag="yo")
                nc.vector.memset(yo, 0.0)
                nc.sync.dma_start(out_d[:], yo)
            for _rep in range(repeat):
                _emit_body(nc, mode, mmdt, mm_ap, wdt, pools, w_d, b_d, out_d,
                           consts, accum=(_rep > 0))

    return nc


def _emit_body(nc, mode, mmdt, mm_ap, wdt, pools, w_d, b_d, out_d, consts,
               accum=False):
    f32 = mybir.dt.float32
    cpool, wpool, spool, xpool, ypool, pspool, ptpool, pgpool = pools
    xt0, gin, gw1, gw2, gw3, gb1, gb2, gb3, emat, bts, ident = consts

    # ---- weight slab DMAs, issued first (DMA is the bottleneck) ----
    wsl = []
    for l in range(3):
        row = []
        for e in range(K):
            t = wpool.tile([P, KSUB, OUTS[l]], wdt, tag="w")
            nc.sync.dma_start(
                t[:, :, : OUTS[l]],
                w_d[l][e].rearrange("(ko p) n -> p ko n", p=P),
            )
            row.append(t)
        wsl.append(row)

    # ---- gating MLP (fp32, exact) ----
    pg1 = pgpool.tile([GH, BS], f32, tag="psg")
    nc.tensor.matmul(pg1, lhsT=gw1, rhs=gin, start=True, stop=True)
    zg1 = ypool.tile([GH, BS], f32, tag="zg1")
    nc.scalar.activation(zg1, pg1, mybir.ActivationFunctionType.Identity,
                         bias=gb1)
    h1 = _elu_from(nc, ypool, zg1, [GH, BS], "g1")

    pg2 = pgpool.tile([GH, BS], f32, tag="psg")
    nc.tensor.matmul(pg2, lhsT=gw2, rhs=h1, start=True, stop=True)
    zg2 = ypool.tile([GH, BS], f32, tag="zg2")
    nc.scalar.activation(zg2, pg2, mybir.ActivationFunctionType.Identity,
                         bias=gb2)
    h2 = _elu_from(nc, ypool, zg2, [GH, BS], "g2")

    pg3 = pgpool.tile([K, BS], f32, tag="psg")
    nc.tensor.matmul(pg3, lhsT=gw3, rhs=h2, start=True, stop=True)
    gT = ypool.tile([K, BS], f32, tag="gT")
    nc.scalar.activation(gT, pg3, mybir.ActivationFunctionType.Identity,
                         bias=gb3)
    if mode == "bf16":
        gT_mm = ypool.tile([K, BS], mmdt, tag="gTmm")
        nc.vector.tensor_copy(gT_mm, gT)
    else:
        gT_mm = gT

    # replicate g across partitions: gTb[p, e, b] = g[b, e]
    pgt = pgpool.tile([P, K, BS], f32, tag="psgtb")
    for e in range(K):
        nc.tensor.matmul(pgt[:, e, :], lhsT=emat[:, e * P:(e + 1) * P],
                         rhs=gT, start=True, stop=True)
    gTb = ypool.tile([P, K, BS], f32, tag="gTb")
    nc.vector.tensor_copy(gTb, pgt)

    # ---- motion layers ----
    # Each layer's output columns are split into two halves so the DVE/ACT
    # post-processing (ELU) and PE transposes of half 0 overlap the PE
    # matmuls of half 1.
    xt = xt0
    sdt = mmdt if mode == "bf16" else f32
    for l in range(3):
        outl = OUTS[l]
        halves = [(0, 256), (256, outl)]

        # per-expert scaled stationaries: one broadcast mult per (expert,
        # k-half) instead of 32 tiny mults
        xk = spool.tile([P, K, KSUB, BS], sdt, tag="xk")
        for e in range(K):
            gslab = gTb[:, e:e + 1, :].to_broadcast((P, 2, BS))
            nc.vector.tensor_tensor(xk[:, e, 0:2, :], xt[:, 0:2, :], gslab,
                                    mybir.AluOpType.mult)
            nc.vector.tensor_tensor(xk[:, e, 2:4, :], xt[:, 2:4, :], gslab,
                                    mybir.AluOpType.mult)

        use_pair = os.environ.get("MANN_PAIR", "1") == "1"
        pss = []
        for h, (lo, hi) in enumerate(halves):
            if use_pair:
                # Two experts run concurrently in disjoint 64-col groups of
                # the PE array (even experts -> psum rows 0:64, odd ->
                # 64:128 via tile_position=(0,64)); summed on DVE after.
                ps_full = pspool.tile([2 * BS, 256], f32, tag=f"psy{h}",
                                      name=f"psy{l}_{h}")
                psA = ps_full[0:BS, : hi - lo]
                psB = ps_full[BS:2 * BS, : hi - lo]
                nc.tensor.matmul(psA, lhsT=mm_ap(gT_mm),
                                 rhs=mm_ap(bts[l][:, lo:hi]),
                                 start=True, stop=False,
                                 skip_group_check=True)
                for e0 in range(0, K, 2):
                    for ks in range(KSUB):
                        last = (e0 == K - 2 and ks == KSUB - 1)
                        nc.tensor.matmul(
                            psA,
                            lhsT=mm_ap(xk[:, e0, ks, :]),
                            rhs=mm_ap(wsl[l][e0][:, ks, lo:hi]),
                            start=False, stop=last,
                            skip_group_check=True,
                        )
                        nc.tensor.matmul(
                            psB,
                            lhsT=mm_ap(xk[:, e0 + 1, ks, :]),
                            rhs=mm_ap(wsl[l][e0 + 1][:, ks, lo:hi]),
                            start=(e0 == 0 and ks == 0), stop=last,
                            tile_position=(0, BS),
                            skip_group_check=True,
                        )
                pss.append((psA, psB))
            else:
                ps_full = pspool.tile([BS, 256], f32, tag=f"psy{h}",
                                      name=f"psy{l}_{h}")
                ps = ps_full[:, : hi - lo]
                nc.tensor.matmul(ps, lhsT=mm_ap(gT_mm),
                                 rhs=mm_ap(bts[l][:, lo:hi]),
                                 start=True, stop=False)
                for e in range(K):
                    for ks in range(KSUB):
                        nc.tensor.matmul(
                            ps,
                            lhsT=mm_ap(xk[:, e, ks, :]),
                            rhs=mm_ap(wsl[l][e][:, ks, lo:hi]),
                            start=False,
                            stop=(e == K - 1 and ks == KSUB - 1),
                        )
                pss.append((ps, None))

        if l < 2:
            ptr = ptpool.tile([P, KSUB, BS], f32, tag="ptr")
            xt_next = xpool.tile([P, KSUB, BS], f32, tag="xtn")
            for h, (lo, hi) in enumerate(halves):
                psA, psB = pss[h]
                if psB is not None:
                    # DVE may read only one PSUM operand per instruction:
                    # copy psB to SBUF first, then add.
                    zb = ypool.tile([BS, hi - lo], f32, tag=f"zb{h}")
                    nc.vector.tensor_copy(zb, psB)
                    z = ypool.tile([BS, hi - lo], f32, tag=f"z{h}")
                    nc.vector.tensor_tensor(z, psA, zb,
                                            mybir.AluOpType.add)
                    src = z
                else:
                    src = psA
                y = _elu_from(nc, ypool, src, [BS, hi - lo], f"ml{h}")
                for c in range(2):
                    nc.tensor.transpose(ptr[:, 2 * h + c, :],
                                        y[:, c * P:(c + 1) * P], ident)
                nc.vector.tensor_copy(xt_next[:, 2 * h:2 * h + 2, :],
                                      ptr[:, 2 * h:2 * h + 2, :])
            xt = xt_next
        else:
            yo = ypool.tile([BS, OUT_DIM], f32, tag="yo")
            for h, (lo, hi) in enumerate(halves):
                psA, psB = pss[h]
                if psB is not None:
                    zb = ypool.tile([BS, hi - lo], f32, tag=f"zb{h}")
                    nc.vector.tensor_copy(zb, psB)
                    nc.vector.tensor_tensor(yo[:, lo:hi], psA, zb,
                                            mybir.AluOpType.add)
                else:
                    nc.vector.tensor_copy(yo[:, lo:hi], psA)
            if accum:
                # benchmark-repeat builds accumulate so no body is dead code
                nc.gpsimd.dma_start(out_d[:], yo,
                                    accum_op=mybir.AluOpType.add)
            else:
                nc.sync.dma_start(out_d[:], yo)


def _get_nc(mode):
    repeat = int(os.environ.get("MANN_BENCH_REPEAT", "1"))
    key = (mode, repeat)
    if key not in _NC_CACHE:
        _NC_CACHE[key] = _build(mode, repeat)
    return _NC_CACHE[key]


def _make_emat():
    e = np.zeros((K, K * P), np.float32)
    for j in range(K):
        e[j, j * P:(j + 1) * P] = 1.0
    return e


def prepare_inputs(x, gating_idx, GW1, Gb1, GW2, Gb2, GW3, Gb3,
                   Wk1, bk1, Wk2, bk2, Wk3, bk3, mode):
    wnp = ml_dtypes.bfloat16 if mode == "bf16" else np.float32
    f32 = np.float32
    x = np.asarray(x, f32)
    idx = np.asarray(gating_idx).astype(np.int64)

    xT = np.zeros((IN_PAD, B), f32)
    xT[:IN_DIM] = x.T
    ginT = np.ascontiguousarray(x[:, idx].T)

    w1 = np.zeros((K, IN_PAD, HID), f32)
    w1[:, :IN_DIM] = np.asarray(Wk1, f32).transpose(0, 2, 1)
    w2 = np.ascontiguousarray(np.asarray(Wk2, f32).transpose(0, 2, 1))
    w3 = np.ascontiguousarray(np.asarray(Wk3, f32).transpose(0, 2, 1))

    shared = {
        "w0": w1.astype(wnp), "w1": w2.astype(wnp), "w2": w3.astype(wnp),
        "b0": np.asarray(bk1, f32).astype(wnp),
        "b1": np.asarray(bk2, f32).astype(wnp),
        "b2": np.asarray(bk3, f32).astype(wnp),
        "gw1": np.asarray(GW1, f32), "gw2": np.asarray(GW2, f32),
        "gw3": np.asarray(GW3, f32),
        "gb1": np.asarray(Gb1, f32).reshape(GH, 1),
        "gb2": np.asarray(Gb2, f32).reshape(GH, 1),
        "gb3": np.asarray(Gb3, f32).reshape(K, 1),
        "emat": _make_emat(),
    }
    in_maps = []
    for c in range(N_CORES):
        m = dict(shared)
        m["xT"] = np.ascontiguousarray(xT[:, c * BS:(c + 1) * BS])
        m["ginT"] = np.ascontiguousarray(ginT[:, c * BS:(c + 1) * BS])
        in_maps.append(m)
    return in_maps


def kernel(**inputs):
    global LAST_RESULTS
    mode = MM_MODE
    nc = _get_nc(mode)
    in_maps = prepare_inputs(mode=mode, **inputs)
    trace = os.environ.get("MANN_TRACE", "0") == "1"
    kwargs = {}
    if trace:
        kwargs["trace"] = True
    res = run_bass_kernel_spmd(nc, in_maps, core_ids=list(range(N_CORES)),
                               **kwargs)
    LAST_RESULTS = res
    out = np.concatenate([r["out"] for r in res.results], axis=0)
    return out.astype(np.float32)
